# revision 1
# baseline (speedup 1.0000x reference)
"""Trainium2 Bass kernel for MultiHead GQA attention (B=1, S=2048, D=1024,
16 q-heads / 4 kv-heads, GQA group 4, RoPE, causal).

Sharding: tensor-parallel over heads. Core c (of 8) computes 2 query heads
{g, g+4} (c even) or {g+8, g+12} (c odd) with g = c//2, which all attend kv
head g (jnp.tile GQA semantics: q-head h uses kv head h % 4). Wq/Wk/Wv are
column-sharded, Wo row-sharded; each core produces a partial [D, S] output
(transposed) and the host reduces the 8 partials, transposes, and adds bo.

Device dataflow (per core, everything in "transposed" [feature, seq] layout
so no on-device transposes of activations are needed):
  qhT[128,S]  = Wq_c.T @ qT   (+bq)   -> RoPE (block-deinterleaved pairs)
  khT[64,S]   = Wk_c.T @ kT   (+bk)   -> RoPE
  vhT[64,S]   = Wv_c.T @ vT   (+bv)   -> PE-transposed to vh[S,64] (+ ones col)
  scoresT[j,i] = khT[:,j].T @ qhT[:,i]          (PE, K=64)
  pT = exp(scoresT/8)       (ACT, causal-masked via precomputed 0/1 tiles)
  o_aug[65,i] += vh_aug[j].T @ pT[j,i]          (PE; row 64 = softmax denom)
  norm: recip of denom (DVE) -> PE broadcast over 64 partitions -> DVE mul
  outT_partial[e,s] = Wo_c[:,e].T @ attnT       (PE)

RoPE trick: the head_dim is permuted on the host (even dims then odd dims)
in Wq/Wk columns, so rotation pairs are partition blocks [0:32)/[32:64) and
the device applies rope with quadrant-aligned copies + two muls + add using
host-precomputed cos / sign-folded sin tables. The permutation cancels in
q.k dot products and does not touch V or Wo.

The causal mask input is not transferred: the reference mask is tril(ones)
and masking is applied structurally (skipped tiles + 4 precomputed diagonal
mask tiles).
"""

import numpy as np
from contextlib import ExitStack

import concourse.bass as bass
from concourse import bacc
import concourse.mybir as mybir
import concourse.tile as tile
from concourse.bass_utils import run_bass_kernel_spmd

f32 = mybir.dt.float32
f32r = mybir.dt.float32r
USE_F32R = False
MDT = f32r if USE_F32R else f32

S = 2048
D = 1024
HEADS = 16
HD = 64
KVH = 4
N_CORES = 8

ST = 512          # i-tile (free dim of most matmuls)
NS = S // ST      # 4
FP = 128          # contraction chunk
NF = D // FP      # 8
JTS = 128         # j-chunk (key positions per score tile partition dim)
NJ = S // JTS     # 16
NE = D // 128     # 8 output-feature chunks

_CACHE = {}


def _build_program():
    if "nc" in _CACHE:
        return _CACHE["nc"]

    nc = bacc.Bacc("TRN2", target_bir_lowering=False, debug=False)

    def din(name, shape, dt=f32):
        return nc.dram_tensor(name, shape, dt, kind="ExternalInput").ap()

    qT = din("qT", [D, S], MDT)
    kT = din("kT", [D, S], MDT)
    vT = din("vT", [D, S], MDT)
    wq = din("wq", [128, NF * 128], MDT)
    wk = din("wk", [128, NF * 64], MDT)
    wv = din("wv", [128, NF * 64], MDT)
    wo = din("wo", [128, D], MDT)
    bq = din("bq", [128, 1])
    bk = din("bk", [64, 1])
    bv = din("bv", [64, 1])
    cosk = din("cosk", [64, S], MDT)
    sink = din("sink", [64, S], MDT)
    cmask = din("cmask", [128, 896], MDT)
    ident_in = din("ident", [64, 64], MDT)
    ones_in = din("ones", [128, 64], MDT)
    outT = nc.dram_tensor("outT", [D, S], f32, kind="ExternalOutput").ap()
    rcb = nc.dram_tensor("rcb", [2, 2 * ST], f32).ap()   # recip bounce (internal)

    Identity = mybir.ActivationFunctionType.Identity
    Exp = mybir.ActivationFunctionType.Exp
    Ln = mybir.ActivationFunctionType.Ln

    from concourse import library_config

    with tile.TileContext(nc) as tc, ExitStack() as ctx:
        const = ctx.enter_context(tc.tile_pool(name="const", bufs=1))
        big = ctx.enter_context(tc.tile_pool(name="big", bufs=1))
        stream = ctx.enter_context(tc.tile_pool(name="stream", bufs=3))
        ptile = ctx.enter_context(tc.tile_pool(name="ptile", bufs=4))
        small = ctx.enter_context(tc.tile_pool(name="small", bufs=2))
        outb = ctx.enter_context(tc.tile_pool(name="outb", bufs=3))
        psum = ctx.enter_context(tc.tile_pool(name="psum", bufs=4, space="PSUM"))

        def mm(out, lhsT, rhs, start, stop):
            nc.tensor.matmul(out, lhsT=lhsT, rhs=rhs, start=start, stop=stop)

        # ---- weights first (PE needs them first) ----
        wq_sb = const.tile([128, NF, 128], MDT)
        nc.sync.dma_start(out=wq_sb, in_=wq.rearrange("p (nf d) -> p nf d", nf=NF))
        wk_sb = const.tile([128, NF, 64], MDT)
        nc.sync.dma_start(out=wk_sb, in_=wk.rearrange("p (nf d) -> p nf d", nf=NF))
        wv_sb = const.tile([128, NF, 64], MDT)
        nc.sync.dma_start(out=wv_sb, in_=wv.rearrange("p (nf d) -> p nf d", nf=NF))
        # constants used later: issue from DVE queue to keep SP free
        wo_sb = const.tile([128, D], MDT)
        nc.scalar.dma_start(out=wo_sb, in_=wo)
        bq_sb = const.tile([128, 1], f32)
        nc.scalar.dma_start(out=bq_sb, in_=bq)
        bk_sb = const.tile([64, 1], f32)
        nc.scalar.dma_start(out=bk_sb, in_=bk)
        bv_sb = const.tile([64, 1], f32)
        nc.scalar.dma_start(out=bv_sb, in_=bv)
        cosk_sb = const.tile([64, S], MDT)
        nc.scalar.dma_start(out=cosk_sb, in_=cosk)
        sink_sb = const.tile([64, S], MDT)
        nc.scalar.dma_start(out=sink_sb, in_=sink)
        ident = const.tile([64, 64], MDT)
        nc.scalar.dma_start(out=ident, in_=ident_in)
        ones4q = const.tile([128, 64], MDT)
        nc.scalar.dma_start(out=ones4q, in_=ones_in)
        # sliding causal band mask: cm_sb[jp, c] = 1.0 iff jp <= c - 384
        cm_sb = const.tile([128, 896], MDT)
        nc.scalar.dma_start(out=cm_sb, in_=cmask)

        # ---- projections ----
        def project(src_dram, w_sb, nd, copies):
            ps = [psum.tile([128, ST], f32, tag="acc", bufs=4, name=f"pj{s}")
                  for s in range(NS)]
            for f in range(NF):
                xin = stream.tile([128, S], MDT, tag="xin", bufs=5)
                if f == 0:
                    # split so the first s-tile lands (and PE starts) sooner
                    for s in range(NS):
                        nc.sync.dma_start(
                            out=xin[:, s * ST:(s + 1) * ST],
                            in_=src_dram[0:FP, s * ST:(s + 1) * ST])
                else:
                    nc.sync.dma_start(out=xin, in_=src_dram[f * FP:(f + 1) * FP, :])
                for s in range(NS):
                    mm(ps[s][0:nd, :], w_sb[:, f, 0:nd],
                       xin[:, s * ST:(s + 1) * ST],
                       start=(f == 0), stop=(f == NF - 1))
            for s in range(NS):
                for (r0, r1, dst, bias_ap) in copies:
                    nc.scalar.activation(
                        out=dst[:, s * ST:(s + 1) * ST], in_=ps[s][r0:r1, :],
                        func=Identity, bias=bias_ap,
                    )

        qh0 = big.tile([64, S], MDT)
        qh1 = big.tile([64, S], MDT)
        khT = big.tile([64, S], MDT)
        vhT = big.tile([64, S], MDT)
        project(qT, wq_sb, 128,
                [(0, 64, qh0, bq_sb[0:64, :]), (64, 128, qh1, bq_sb[64:128, :])])
        project(kT, wk_sb, 64, [(0, 64, khT, bk_sb)])
        project(vT, wv_sb, 64, [(0, 64, vhT, bv_sb)])

        # ---- RoPE (in-place; pairs are partition blocks [0:32)/[32:64)) ----
        def rope64(x, nm):
            swap = stream.tile([64, S], MDT, tag="swap", name=f"swap_{nm}", bufs=2)
            for (srcp, dstp) in ((32, 0), (0, 32)):
                nc.vector.tensor_copy(swap[dstp:dstp + 32, :], x[srcp:srcp + 32, :])
            nc.vector.tensor_mul(x, x, cosk_sb)
            nc.vector.tensor_mul(swap, swap, sink_sb)
            nc.vector.tensor_add(x, x, swap)

        rope64(khT, "k")
        rope64(qh0, "q0")
        rope64(qh1, "q1")

        # ---- transpose V to [seq, dim] (+ ones column for softmax denom) ----
        vh_aug = big.tile([128, NJ, 65], MDT)
        nc.scalar.copy(vh_aug[:, :, 64], ones4q[0:128, 0:NJ])
        with nc.allow_low_precision(reason="transpose psum, same width as f32"):
            for jt in range(NJ):
                tp = psum.tile([128, 64], MDT, tag="mm", bufs=4, name="tp")
                nc.tensor.transpose(tp, vhT[:, jt * JTS:(jt + 1) * JTS], ident)
                nc.scalar.copy(vh_aug[:, jt, 0:64], tp)

        # ---- attention (2 heads share khT / vh_aug) ----
        attn = big.tile([128, S], MDT)
        for h in range(2):
            q_sl = (qh0, qh1)[h]
            po = [psum.tile([65, ST], f32, tag="acc", bufs=4, name=f"po{s}")
                  for s in range(NS)]

            def finish_pair(p):
                # its (2p, 2p+1) accumulators are complete: normalize + write
                its = (2 * p, 2 * p + 1)
                sums = small.tile([1, 2 * ST], f32, tag="sums", name=f"sums{h}{p}")
                rc = small.tile([1, 2 * ST], f32, tag="rc", name=f"rc{h}{p}")
                lns = small.tile([1, 2 * ST], f32, tag="lns", name=f"lns{h}{p}")
                for it in its:
                    nc.scalar.copy(sums[0:1, ST * (it % 2):ST * (it % 2) + ST],
                                   po[it][64:65, :])
                nc.scalar.activation(out=lns, in_=sums, func=Ln)
                nc.scalar.activation(out=rc, in_=lns, func=Exp, scale=-1.0)
                nc.sync.dma_start(out=rcb[h:h + 1, :], in_=rc)
                for it in its:
                    a_sl = attn[h * 64:(h + 1) * 64, it * ST:(it + 1) * ST]
                    nc.scalar.copy(a_sl, po[it][0:64, :])
                    bct = ptile.tile([128, ST], f32, tag="bct", bufs=2, name="bct")
                    bslc = bct[h * 64:(h + 1) * 64, :]
                    rsrc = rcb[h:h + 1, ST * (it % 2):ST * (it % 2) + ST]
                    rsrc = bass.AP(tensor=rsrc.tensor, offset=rsrc.offset,
                                   ap=[[0, 64]] + list(rsrc.ap)[1:])
                    nc.sync.dma_start(out=bslc, in_=rsrc)
                    nc.vector.tensor_mul(a_sl, a_sl, bslc)

            for jt in range(NJ):
                it0 = jt // 4
                for it in range(it0, NS):
                    # on the diagonal i-tile, columns below the diagonal are
                    # fully masked: skip them in scores/exp/PV entirely
                    lo = (jt - 4 * it) * JTS if it == it0 else 0
                    ps = psum.tile([128, ST], f32, tag="mm", bufs=4)
                    mm(ps[:, lo:], khT[:, jt * JTS:(jt + 1) * JTS],
                       q_sl[:, it * ST + lo:(it + 1) * ST], start=True, stop=True)
                    pt = ptile.tile([128, ST], MDT, tag="pt", bufs=6)
                    nc.scalar.activation(out=pt[:, lo:], in_=ps[:, lo:],
                                         func=Exp, scale=0.125)
                    if it == it0:
                        # partial band: keep iff jp <= (i_off - lo)
                        nc.vector.tensor_mul(pt[:, lo:lo + JTS],
                                             pt[:, lo:lo + JTS],
                                             cm_sb[:, 384:512])
                    mm(po[it][:, lo:], vh_aug[:, jt, :], pt[:, lo:],
                       start=(jt == 0), stop=(jt == 4 * it + 3))
                if jt == 7:
                    finish_pair(0)
            finish_pair(1)

        # ---- output projection (partial over this core's 128 dims) ----
        for it in range(NS):
            for e in range(NE):
                pw = psum.tile([128, ST], f32, tag="mm", bufs=4, name="pw")
                mm(pw, wo_sb[:, e * 128:(e + 1) * 128],
                   attn[:, it * ST:(it + 1) * ST], start=True, stop=True)
                ob = outb.tile([128, ST], f32, tag="ob")
                nc.vector.tensor_copy(ob, pw)
                nc.gpsimd.dma_start(
                    out=outT[e * 128:(e + 1) * 128, it * ST:(it + 1) * ST], in_=ob
                )

    nc.compile()
    _CACHE["nc"] = nc
    return nc


def _host_tables():
    if "tables" in _CACHE:
        return _CACHE["tables"]
    # faithful to reference: exp = -2*arange(0,64,2)/64
    expv = -2.0 * np.arange(0, HD, 2, dtype=np.float32) / HD
    thetas = np.power(np.float32(10000.0), expv).astype(np.float32)    # [32]
    m = np.arange(S, dtype=np.float32)
    freq = np.outer(m, thetas).astype(np.float32)                      # [S, 32]
    cos = np.cos(freq).astype(np.float32).T                            # [32, S]
    sin = np.sin(freq).astype(np.float32).T
    cos64 = np.concatenate([cos, cos], 0)                              # [64, S]
    sin64 = np.concatenate([-sin, sin], 0)                             # [64, S]
    cos64 = np.ascontiguousarray(cos64)
    sin64 = np.ascontiguousarray(sin64)
    perm = np.concatenate([np.arange(0, HD, 2), np.arange(1, HD, 2)])  # deinterleave
    slide = (np.arange(128)[:, None] <= (np.arange(896)[None, :] - 384))
    slide = np.ascontiguousarray(slide.astype(np.float32))
    _CACHE["tables"] = (cos64, sin64, perm, slide)
    return _CACHE["tables"]


def kernel(**inputs):
    q = np.asarray(inputs["q"], np.float32)[0]       # [S, D]
    k = np.asarray(inputs["k"], np.float32)[0]
    v = np.asarray(inputs["v"], np.float32)[0]
    Wq = np.asarray(inputs["Wq"], np.float32)
    Wk = np.asarray(inputs["Wk"], np.float32)
    Wv = np.asarray(inputs["Wv"], np.float32)
    Wo = np.asarray(inputs["Wo"], np.float32)
    bq = np.asarray(inputs["bq"], np.float32)
    bk = np.asarray(inputs["bk"], np.float32)
    bv = np.asarray(inputs["bv"], np.float32)
    bo = np.asarray(inputs["bo"], np.float32)

    cos64, sin64, perm, slide = _host_tables()

    # head_dim deinterleave permutation applied to q/k projection columns
    permQ = np.concatenate([h * HD + perm for h in range(HEADS)])
    permK = np.concatenate([g * HD + perm for g in range(KVH)])
    Wqp = Wq[:, permQ]
    bqp = bq[permQ]
    Wkp = Wk[:, permK]
    bkp = bk[permK]

    qT = np.ascontiguousarray(q.T)
    kT = np.ascontiguousarray(k.T)
    vT = np.ascontiguousarray(v.T)

    in_maps = []
    head_pairs = []
    for c in range(N_CORES):
        g = c // 2
        if c % 2 == 0:
            h0, h1 = g, g + 4
        else:
            h0, h1 = g + 8, g + 12
        head_pairs.append((h0, h1))
        wq_c = np.ascontiguousarray(
            np.concatenate([Wqp[:, h0 * HD:(h0 + 1) * HD],
                            Wqp[:, h1 * HD:(h1 + 1) * HD]], axis=1))
        bq_c = np.ascontiguousarray(
            np.concatenate([bqp[h0 * HD:(h0 + 1) * HD],
                            bqp[h1 * HD:(h1 + 1) * HD]]).reshape(128, 1))
        wo_c = np.ascontiguousarray(
            np.concatenate([Wo[h0 * HD:(h0 + 1) * HD, :],
                            Wo[h1 * HD:(h1 + 1) * HD, :]], axis=0))
        def warr(w):
            # [1024, nd] -> [128, NF*nd] with chunk-of-128-rows as middle dim
            nd = w.shape[1]
            return np.ascontiguousarray(
                w.reshape(NF, FP, nd).transpose(1, 0, 2).reshape(FP, NF * nd))

        in_maps.append({
            "qT": qT, "kT": kT, "vT": vT,
            "wq": warr(wq_c),
            "wk": warr(Wkp[:, g * HD:(g + 1) * HD]),
            "wv": warr(Wv[:, g * HD:(g + 1) * HD]),
            "wo": wo_c,
            "bq": bq_c,
            "bk": np.ascontiguousarray(bkp[g * HD:(g + 1) * HD].reshape(64, 1)),
            "bv": np.ascontiguousarray(bv[g * HD:(g + 1) * HD].reshape(64, 1)),
            "cosk": cos64, "sink": sin64, "cmask": slide,
            "ident": np.eye(64, dtype=np.float32),
            "ones": np.ones((128, 64), np.float32),
        })

    nc = _build_program()
    res = run_bass_kernel_spmd(nc, in_maps, list(range(N_CORES)))
    acc = np.zeros((D, S), np.float32)
    for r in res.results:
        acc += np.asarray(r["outT"], np.float32)
    out = acc.T + bo[None, :]
    return out[None].astype(np.float32)



# revision 16
# speedup vs baseline: 1.4211x; 1.4211x over previous
"""Trainium2 Bass kernel for MultiHead GQA attention (B=1, S=2048, D=1024,
16 q-heads / 4 kv-heads, GQA group 4, RoPE, causal).

Sharding: tensor-parallel over heads. Core c (of 8) computes 2 query heads
{g, g+4} (c even) or {g+8, g+12} (c odd) with g = c//2, which all attend kv
head g (jnp.tile GQA semantics: q-head h uses kv head h % 4). Wq/Wk/Wv are
column-sharded, Wo row-sharded; each core produces a partial [D, S] output
(transposed, bf16) and the host reduces the 8 partials, transposes, adds bo.

All matmul datapaths run in bf16 (PE 1 cycle/row vs 4 for fp32), with fp32
PSUM accumulation. Device dataflow per core ([feature, seq] layout):
  qhT[128,S]  = Wq_c.T @ qT   (+bq)   -> RoPE (block-deinterleaved pairs)
  khT[64,S]   = Wk_c.T @ kT   (+bk)   -> RoPE
  vhT[64,S]   = Wv_c.T @ vT   (+bv)   -> PE-transposed to vh[S,64] (+ ones col)
  scoresT[j,i] = khT[:,j].T @ qhT[:,i]          (PE, K=64)
  pT = exp(scoresT/8)  bf16 (ACT, causal via skipped tiles + diag mask tile)
  o_aug[65,i] += vh_aug[j].T @ pT[j,i]          (PE; row 64 = softmax denom)
  rc = reciprocal_approx_fast(denom)  (DVE, reads PSUM directly)
  PE broadcast rc over 64 partitions (ones[1,64].T @ rc, f32r) -> DVE mul
  outT_partial[e,s] = Wo_c[:,e].T @ attnT       (PE) -> bf16 -> DMA out

RoPE trick: head_dim is permuted on the host (even dims then odd dims) in
Wq/Wk columns so rotation pairs are partition blocks [0:32)/[32:64); device
applies rope with copies + two muls + add using host-precomputed cos /
sign-folded sin tables. The permutation cancels in q.k dots.

The causal mask input is never transferred: reference mask is tril(ones),
applied structurally (skipped tiles + one precomputed [128,128] tril tile).
"""

import numpy as np
import ml_dtypes
from contextlib import ExitStack

import concourse.bass as bass
from concourse import bacc
import concourse.mybir as mybir
import concourse.tile as tile
from concourse.bass_utils import run_bass_kernel_spmd

f32 = mybir.dt.float32
f32r = mybir.dt.float32r
bf16 = mybir.dt.bfloat16
MDT = bf16
NPDT = ml_dtypes.bfloat16

S = 2048
D = 1024
HEADS = 16
HD = 64
KVH = 4
N_CORES = 8

ST = 512          # i-tile (free dim of most matmuls)
NS = S // ST      # 4
FP = 128          # contraction chunk
NF = D // FP      # 8
JTS = 128         # j-chunk (key positions per score tile partition dim)
NJ = S // JTS     # 16
NE = D // 128     # 8 output-feature chunks

_CACHE = {}


def _build_program():
    if "nc" in _CACHE:
        return _CACHE["nc"]

    nc = bacc.Bacc("TRN2", target_bir_lowering=False, debug=False)

    def din(name, shape, dt=MDT):
        return nc.dram_tensor(name, shape, dt, kind="ExternalInput").ap()

    qT = din("qT", [D, S])
    kT = din("kT", [D, S])
    vT = din("vT", [D, S])
    wq = din("wq", [128, NF * 128])
    wk = din("wk", [128, NF * 64])
    wv = din("wv", [128, NF * 64])
    wo = din("wo", [128, D])
    bq = din("bq", [128, 1], f32)
    bk = din("bk", [64, 1], f32)
    bv = din("bv", [64, 1], f32)
    cosk = din("cosk", [64, S])
    sink = din("sink", [64, S])
    cmask = din("cmask", [128, 128])
    ident_in = din("ident", [64, 64], f32)
    ones_in = din("ones", [128, 128])
    outT = nc.dram_tensor("outT", [D, S], MDT, kind="ExternalOutput").ap()

    Identity = mybir.ActivationFunctionType.Identity
    Exp = mybir.ActivationFunctionType.Exp

    with tile.TileContext(nc) as tc, ExitStack() as ctx:
        const = ctx.enter_context(tc.tile_pool(name="const", bufs=1))
        big = ctx.enter_context(tc.tile_pool(name="big", bufs=1))
        stream = ctx.enter_context(tc.tile_pool(name="stream", bufs=3))
        ptile = ctx.enter_context(tc.tile_pool(name="ptile", bufs=4))
        small = ctx.enter_context(tc.tile_pool(name="small", bufs=2))
        outb = ctx.enter_context(tc.tile_pool(name="outb", bufs=3))
        psum = ctx.enter_context(tc.tile_pool(name="psum", bufs=4, space="PSUM"))

        def mm(out, lhsT, rhs, start, stop):
            nc.tensor.matmul(out, lhsT=lhsT, rhs=rhs, start=start, stop=stop)

        # ---- weights first (PE needs them first) ----
        wq_sb = const.tile([128, NF, 128], MDT)
        nc.sync.dma_start(out=wq_sb, in_=wq.rearrange("p (nf d) -> p nf d", nf=NF))
        wk_sb = const.tile([128, NF, 64], MDT)
        nc.sync.dma_start(out=wk_sb, in_=wk.rearrange("p (nf d) -> p nf d", nf=NF))
        wv_sb = const.tile([128, NF, 64], MDT)
        nc.sync.dma_start(out=wv_sb, in_=wv.rearrange("p (nf d) -> p nf d", nf=NF))
        # constants used later: issue from scalar queue to keep SP free
        wo_sb = const.tile([128, D], MDT)
        nc.scalar.dma_start(out=wo_sb, in_=wo)
        bq_sb = const.tile([128, 1], f32)
        nc.scalar.dma_start(out=bq_sb, in_=bq)
        bk_sb = const.tile([64, 1], f32)
        nc.scalar.dma_start(out=bk_sb, in_=bk)
        bv_sb = const.tile([64, 1], f32)
        nc.scalar.dma_start(out=bv_sb, in_=bv)
        cosk_sb = const.tile([64, S], MDT)
        nc.scalar.dma_start(out=cosk_sb, in_=cosk)
        sink_sb = const.tile([64, S], MDT)
        nc.scalar.dma_start(out=sink_sb, in_=sink)
        ident = const.tile([64, 64], f32)
        nc.scalar.dma_start(out=ident, in_=ident_in)
        ones4q = const.tile([128, 128], MDT)
        nc.scalar.dma_start(out=ones4q, in_=ones_in)
        # diagonal causal tile: cm_sb[jp, c] = 1.0 iff jp <= c
        cm_sb = const.tile([128, 128], MDT)
        nc.scalar.dma_start(out=cm_sb, in_=cmask)

        # ---- projections ----
        def project(src_dram, w_sb, nd, copies, dma_eng):
            ps = [psum.tile([128, ST], f32, tag="acc", bufs=4, name=f"pj{s}")
                  for s in range(NS)]
            for f in range(NF):
                xin = stream.tile([128, S], MDT, tag="xin", bufs=5)
                if f == 0:
                    # split so the first s-tile lands (and PE starts) sooner
                    for s in range(NS):
                        dma_eng.dma_start(
                            out=xin[:, s * ST:(s + 1) * ST],
                            in_=src_dram[0:FP, s * ST:(s + 1) * ST])
                else:
                    dma_eng.dma_start(out=xin, in_=src_dram[f * FP:(f + 1) * FP, :])
                for s in range(NS):
                    mm(ps[s][0:nd, :], w_sb[:, f, 0:nd],
                       xin[:, s * ST:(s + 1) * ST],
                       start=(f == 0), stop=(f == NF - 1))
            for s in range(NS):
                for (r0, r1, dst, bias_ap) in copies:
                    nc.scalar.activation(
                        out=dst[:, s * ST:(s + 1) * ST], in_=ps[s][r0:r1, :],
                        func=Identity, bias=bias_ap,
                    )

        qh0 = big.tile([64, S], MDT)
        qh1 = big.tile([64, S], MDT)
        khT = big.tile([64, S], MDT)
        vhT = big.tile([64, S], f32)   # f32: PE transpose needs 4-byte PSUM
        project(qT, wq_sb, 128,
                [(0, 64, qh0, bq_sb[0:64, :]), (64, 128, qh1, bq_sb[64:128, :])],
                nc.sync)
        project(kT, wk_sb, 64, [(0, 64, khT, bk_sb)], nc.sync)
        project(vT, wv_sb, 64, [(0, 64, vhT, bv_sb)], nc.gpsimd)

        # ---- RoPE (in-place; pairs are partition blocks [0:32)/[32:64)) ----
        def rope64(x, nm):
            swap = stream.tile([64, S], MDT, tag="swap", name=f"swap_{nm}", bufs=2)
            for (srcp, dstp) in ((32, 0), (0, 32)):
                nc.vector.tensor_copy(swap[dstp:dstp + 32, :], x[srcp:srcp + 32, :])
            nc.vector.tensor_mul(x, x, cosk_sb)
            nc.vector.tensor_mul(swap, swap, sink_sb)
            nc.vector.tensor_add(x, x, swap)

        rope64(khT, "k")
        rope64(qh0, "q0")
        rope64(qh1, "q1")

        # ---- transpose V to [seq, dim] (+ ones column for softmax denom) ----
        vh_aug = big.tile([128, NJ, 65], MDT)
        nc.scalar.copy(vh_aug[:, :, 64], ones4q[0:128, 0:NJ])
        with nc.allow_low_precision(reason="transpose psum, same width as f32"):
            for jt in range(NJ):
                tp = psum.tile([128, 64], f32, tag="mm", bufs=4, name="tp")
                nc.tensor.transpose(tp, vhT[:, jt * JTS:(jt + 1) * JTS], ident)
                nc.scalar.copy(vh_aug[:, jt, 0:64], tp)

        # ---- attention (2 heads share khT / vh_aug) ----
        attn = big.tile([128, S], MDT)
        for h in range(2):
            q_sl = (qh0, qh1)[h]
            po = [psum.tile([65, ST], f32, tag="acc", bufs=4, name=f"po{s}")
                  for s in range(NS)]

            def finish_pair(p):
                # its (2p, 2p+1) accumulators are complete: normalize + write
                its = (2 * p, 2 * p + 1)
                # denom row -> partition 0 (ACT crossbar), recip on DVE,
                # bf16 cast, then PE-broadcast to 128 rows (K=1 matmul).
                sums = small.tile([1, 2 * ST], f32, tag="sums", name=f"sums{h}{p}")
                rc = small.tile([1, 2 * ST], f32, tag="rc", name=f"rc{h}{p}")
                rcb = small.tile([1, 2 * ST], MDT, tag="rcb", name=f"rcb{h}{p}")
                for it in its:
                    c0 = ST * (it % 2)
                    nc.scalar.copy(sums[0:1, c0:c0 + ST], po[it][64:65, :])
                nc.vector.reciprocal_approx_fast(out=rc, in_=sums)
                nc.vector.tensor_copy(rcb, rc)
                for it in its:
                    c0 = ST * (it % 2)
                    bct = psum.tile([128, ST], f32, tag="mm", bufs=4, name="bct")
                    nc.tensor.matmul(
                        bct, lhsT=ones4q[0:1, 0:128],
                        rhs=rcb[0:1, c0:c0 + ST],
                        start=True, stop=True)
                    a_sl = attn[h * 64:(h + 1) * 64, it * ST:(it + 1) * ST]
                    nc.scalar.copy(a_sl, po[it][0:64, :])
                    nc.vector.tensor_mul(a_sl, a_sl,
                                         bct[h * 64:(h + 1) * 64, :])

            for jt in range(NJ):
                it0 = jt // 4
                for it in range(it0, NS):
                    # on the diagonal i-tile, columns below the diagonal are
                    # fully masked: skip them in scores/exp/PV entirely
                    lo = (jt - 4 * it) * JTS if it == it0 else 0
                    ps = psum.tile([128, ST], f32, tag="mm", bufs=4)
                    mm(ps[:, lo:], khT[:, jt * JTS:(jt + 1) * JTS],
                       q_sl[:, it * ST + lo:(it + 1) * ST], start=True, stop=True)
                    pt = ptile.tile([128, ST], MDT, tag="pt", bufs=6)
                    nc.scalar.activation(out=pt[:, lo:], in_=ps[:, lo:],
                                         func=Exp, scale=0.125)
                    if it == it0:
                        # partial band: keep iff jp <= c within the tile
                        nc.vector.tensor_mul(pt[:, lo:lo + JTS],
                                             pt[:, lo:lo + JTS],
                                             cm_sb)
                    mm(po[it][:, lo:], vh_aug[:, jt, :], pt[:, lo:],
                       start=(jt == 0), stop=(jt == 4 * it + 3))
                if jt == 7:
                    finish_pair(0)
            finish_pair(1)

        # ---- output projection (partial over this core's 128 dims) ----
        for it in range(NS):
            for e in range(NE):
                pw = psum.tile([128, ST], f32, tag="mm", bufs=4, name="pw")
                mm(pw, wo_sb[:, e * 128:(e + 1) * 128],
                   attn[:, it * ST:(it + 1) * ST], start=True, stop=True)
                ob = outb.tile([128, ST], MDT, tag="ob")
                nc.vector.tensor_copy(ob, pw)
                nc.gpsimd.dma_start(
                    out=outT[e * 128:(e + 1) * 128, it * ST:(it + 1) * ST], in_=ob
                )

    nc.compile()
    _CACHE["nc"] = nc
    return nc


def _host_tables():
    if "tables" in _CACHE:
        return _CACHE["tables"]
    # faithful to reference: exp = -2*arange(0,64,2)/64
    expv = -2.0 * np.arange(0, HD, 2, dtype=np.float32) / HD
    thetas = np.power(np.float32(10000.0), expv).astype(np.float32)    # [32]
    m = np.arange(S, dtype=np.float32)
    freq = np.outer(m, thetas).astype(np.float32)                      # [S, 32]
    cos = np.cos(freq).astype(np.float32).T                            # [32, S]
    sin = np.sin(freq).astype(np.float32).T
    cos64 = np.concatenate([cos, cos], 0)                              # [64, S]
    sin64 = np.concatenate([-sin, sin], 0)                             # [64, S]
    cos64 = np.ascontiguousarray(cos64).astype(NPDT)
    sin64 = np.ascontiguousarray(sin64).astype(NPDT)
    perm = np.concatenate([np.arange(0, HD, 2), np.arange(1, HD, 2)])  # deinterleave
    tril = (np.arange(128)[:, None] <= np.arange(128)[None, :])
    tril = np.ascontiguousarray(tril.astype(NPDT))
    _CACHE["tables"] = (cos64, sin64, perm, tril)
    return _CACHE["tables"]


def kernel(**inputs):
    q = np.asarray(inputs["q"], np.float32)[0]       # [S, D]
    k = np.asarray(inputs["k"], np.float32)[0]
    v = np.asarray(inputs["v"], np.float32)[0]
    Wq = np.asarray(inputs["Wq"], np.float32)
    Wk = np.asarray(inputs["Wk"], np.float32)
    Wv = np.asarray(inputs["Wv"], np.float32)
    Wo = np.asarray(inputs["Wo"], np.float32)
    bq = np.asarray(inputs["bq"], np.float32)
    bk = np.asarray(inputs["bk"], np.float32)
    bv = np.asarray(inputs["bv"], np.float32)
    bo = np.asarray(inputs["bo"], np.float32)

    cos64, sin64, perm, tril = _host_tables()

    # head_dim deinterleave permutation applied to q/k projection columns
    permQ = np.concatenate([h * HD + perm for h in range(HEADS)])
    permK = np.concatenate([g * HD + perm for g in range(KVH)])
    Wqp = Wq[:, permQ]
    bqp = bq[permQ]
    Wkp = Wk[:, permK]
    bkp = bk[permK]

    qT = np.ascontiguousarray(q.T).astype(NPDT)
    kT = np.ascontiguousarray(k.T).astype(NPDT)
    vT = np.ascontiguousarray(v.T).astype(NPDT)

    in_maps = []
    for c in range(N_CORES):
        g = c // 2
        if c % 2 == 0:
            h0, h1 = g, g + 4
        else:
            h0, h1 = g + 8, g + 12
        wq_c = np.ascontiguousarray(
            np.concatenate([Wqp[:, h0 * HD:(h0 + 1) * HD],
                            Wqp[:, h1 * HD:(h1 + 1) * HD]], axis=1))
        bq_c = np.ascontiguousarray(
            np.concatenate([bqp[h0 * HD:(h0 + 1) * HD],
                            bqp[h1 * HD:(h1 + 1) * HD]]).reshape(128, 1))
        wo_c = np.ascontiguousarray(
            np.concatenate([Wo[h0 * HD:(h0 + 1) * HD, :],
                            Wo[h1 * HD:(h1 + 1) * HD, :]], axis=0))

        def warr(w):
            # [1024, nd] -> [128, NF*nd] with chunk-of-128-rows as middle dim
            nd = w.shape[1]
            return np.ascontiguousarray(
                w.reshape(NF, FP, nd).transpose(1, 0, 2).reshape(FP, NF * nd)
            ).astype(NPDT)

        in_maps.append({
            "qT": qT, "kT": kT, "vT": vT,
            "wq": warr(wq_c),
            "wk": warr(Wkp[:, g * HD:(g + 1) * HD]),
            "wv": warr(Wv[:, g * HD:(g + 1) * HD]),
            "wo": wo_c.astype(NPDT),
            "bq": bq_c,
            "bk": np.ascontiguousarray(bkp[g * HD:(g + 1) * HD].reshape(64, 1)),
            "bv": np.ascontiguousarray(bv[g * HD:(g + 1) * HD].reshape(64, 1)),
            "cosk": cos64, "sink": sin64, "cmask": tril,
            "ident": np.eye(64, dtype=np.float32),
            "ones": np.ones((128, 128), NPDT),
        })

    nc = _build_program()
    res = run_bass_kernel_spmd(nc, in_maps, list(range(N_CORES)))
    acc = np.zeros((D, S), np.float32)
    for r in res.results:
        acc += np.asarray(r["outT"], np.float32)
    out = acc.T + bo[None, :]
    return out[None].astype(np.float32)


# revision 20
# speedup vs baseline: 1.5089x; 1.0617x over previous
"""Trainium2 Bass kernel for MultiHead GQA attention (B=1, S=2048, D=1024,
16 q-heads / 4 kv-heads, GQA group 4, RoPE, causal).

Sharding: tensor-parallel over heads. Core c (of 8) computes 2 query heads
{g, g+4} (c even) or {g+8, g+12} (c odd) with g = c//2, which all attend kv
head g (jnp.tile GQA semantics: q-head h uses kv head h % 4). Wq/Wk/Wv are
column-sharded, Wo row-sharded; each core produces a partial [D, S] output
(transposed, bf16) and the host reduces the 8 partials, transposes, adds bo.

All matmul datapaths run in bf16 (PE 1 cycle/row vs 4 for fp32), with fp32
PSUM accumulation. Device dataflow per core ([feature, seq] layout):
  qhT[128,S]  = Wq_c.T @ qT   (+bq)   -> RoPE (block-deinterleaved pairs)
  khT[64,S]   = Wk_c.T @ kT   (+bk)   -> RoPE
  vhT[64,S]   = Wv_c.T @ vT   (+bv)   -> PE-transposed to vh[S,64] (+ ones col)
  scoresT[j,i] = khT[:,j].T @ qhT[:,i]          (PE, K=64)
  pT = exp(scoresT/8)  bf16 (ACT, causal via skipped tiles + diag mask tile)
  o_aug[65,i] += vh_aug[j].T @ pT[j,i]          (PE; row 64 = softmax denom)
  rc = reciprocal_approx_fast(denom)  (DVE, reads PSUM directly)
  PE broadcast rc over 64 partitions (ones[1,64].T @ rc, f32r) -> DVE mul
  outT_partial[e,s] = Wo_c[:,e].T @ attnT       (PE) -> bf16 -> DMA out

RoPE trick: head_dim is permuted on the host (even dims then odd dims) in
Wq/Wk columns so rotation pairs are partition blocks [0:32)/[32:64); device
applies rope with copies + two muls + add using host-precomputed cos /
sign-folded sin tables. The permutation cancels in q.k dots.

The causal mask input is never transferred: reference mask is tril(ones),
applied structurally (skipped tiles + one precomputed [128,128] tril tile).
"""

import numpy as np
import ml_dtypes
from contextlib import ExitStack

import concourse.bass as bass
from concourse import bacc
import concourse.mybir as mybir
import concourse.tile as tile
from concourse.bass_utils import run_bass_kernel_spmd

f32 = mybir.dt.float32
f32r = mybir.dt.float32r
bf16 = mybir.dt.bfloat16
MDT = bf16
NPDT = ml_dtypes.bfloat16

S = 2048
D = 1024
HEADS = 16
HD = 64
KVH = 4
N_CORES = 8

ST = 512          # i-tile (free dim of most matmuls)
NS = S // ST      # 4
FP = 128          # contraction chunk
NF = D // FP      # 8
JTS = 128         # j-chunk (key positions per score tile partition dim)
NJ = S // JTS     # 16
NE = D // 128     # 8 output-feature chunks

_CACHE = {}


def _build_program():
    if "nc" in _CACHE:
        return _CACHE["nc"]

    nc = bacc.Bacc("TRN2", target_bir_lowering=False, debug=False)

    def din(name, shape, dt=MDT):
        return nc.dram_tensor(name, shape, dt, kind="ExternalInput").ap()

    qT = din("qT", [D, S])
    kT = din("kT", [D, S])
    vT = din("vT", [D, S])
    wq = din("wq", [128, NF * 128])
    wk = din("wk", [128, NF * 64])
    wv = din("wv", [128, NF * 64])
    wo = din("wo", [128, D])
    bq = din("bq", [128, 1], f32)
    bk = din("bk", [64, 1], f32)
    bv = din("bv", [64, 1], f32)
    cosk = din("cosk", [64, S])
    sink = din("sink", [64, S])
    cmask = din("cmask", [128, 128])
    ident_in = din("ident", [64, 64], f32)
    ones_in = din("ones", [128, 128])
    outT = nc.dram_tensor("outT", [D, S], MDT, kind="ExternalOutput").ap()
    rcb_d = nc.dram_tensor("rcb_d", [4, 2 * ST], f32).ap()   # recip bounce

    Identity = mybir.ActivationFunctionType.Identity
    Exp = mybir.ActivationFunctionType.Exp

    with tile.TileContext(nc) as tc, ExitStack() as ctx:
        const = ctx.enter_context(tc.tile_pool(name="const", bufs=1))
        big = ctx.enter_context(tc.tile_pool(name="big", bufs=1))
        stream = ctx.enter_context(tc.tile_pool(name="stream", bufs=3))
        ptile = ctx.enter_context(tc.tile_pool(name="ptile", bufs=4))
        small = ctx.enter_context(tc.tile_pool(name="small", bufs=2))
        outb = ctx.enter_context(tc.tile_pool(name="outb", bufs=3))
        psum = ctx.enter_context(tc.tile_pool(name="psum", bufs=4, space="PSUM"))

        def mm(out, lhsT, rhs, start, stop):
            nc.tensor.matmul(out, lhsT=lhsT, rhs=rhs, start=start, stop=stop)

        # ---- weights first (PE needs wq + first q chunk soonest) ----
        wq_sb = const.tile([128, NF, 128], MDT)
        nc.sync.dma_start(out=wq_sb, in_=wq.rearrange("p (nf d) -> p nf d", nf=NF))
        # wk/wv on the scalar queue so the sync queue goes straight to q data
        wk_sb = const.tile([128, NF, 64], MDT)
        nc.scalar.dma_start(out=wk_sb, in_=wk.rearrange("p (nf d) -> p nf d", nf=NF))
        wv_sb = const.tile([128, NF, 64], MDT)
        nc.scalar.dma_start(out=wv_sb, in_=wv.rearrange("p (nf d) -> p nf d", nf=NF))
        bq_sb = const.tile([128, 1], f32)
        nc.scalar.dma_start(out=bq_sb, in_=bq)
        bk_sb = const.tile([64, 1], f32)
        nc.scalar.dma_start(out=bk_sb, in_=bk)
        bv_sb = const.tile([64, 1], f32)
        nc.scalar.dma_start(out=bv_sb, in_=bv)
        cosk_sb = const.tile([64, S], MDT)
        nc.scalar.dma_start(out=cosk_sb, in_=cosk)
        sink_sb = const.tile([64, S], MDT)
        nc.scalar.dma_start(out=sink_sb, in_=sink)
        ident = const.tile([64, 64], f32)
        nc.scalar.dma_start(out=ident, in_=ident_in)
        ones4q = const.tile([128, 128], MDT)
        nc.scalar.dma_start(out=ones4q, in_=ones_in)
        # diagonal causal tile: cm_sb[jp, c] = 1.0 iff jp <= c
        cm_sb = const.tile([128, 128], MDT)
        nc.scalar.dma_start(out=cm_sb, in_=cmask)
        wo_sb = const.tile([128, D], MDT)
        nc.scalar.dma_start(out=wo_sb, in_=wo)

        # ---- projections ----
        def project(src_dram, w_sb, nd, copies, dma_eng):
            ps = [psum.tile([128, ST], f32, tag="acc", bufs=4, name=f"pj{s}")
                  for s in range(NS)]
            for f in range(NF):
                xin = stream.tile([128, S], MDT, tag="xin", bufs=5)
                if f == 0:
                    # split so the first s-tile lands (and PE starts) sooner
                    for s in range(NS):
                        dma_eng.dma_start(
                            out=xin[:, s * ST:(s + 1) * ST],
                            in_=src_dram[0:FP, s * ST:(s + 1) * ST])
                else:
                    dma_eng.dma_start(out=xin, in_=src_dram[f * FP:(f + 1) * FP, :])
                for s in range(NS):
                    mm(ps[s][0:nd, :], w_sb[:, f, 0:nd],
                       xin[:, s * ST:(s + 1) * ST],
                       start=(f == 0), stop=(f == NF - 1))
            for s in range(NS):
                for (r0, r1, dst, bias_ap) in copies:
                    nc.scalar.activation(
                        out=dst[:, s * ST:(s + 1) * ST], in_=ps[s][r0:r1, :],
                        func=Identity, bias=bias_ap,
                    )

        qh0 = big.tile([64, S], MDT)
        qh1 = big.tile([64, S], MDT)
        khT = big.tile([64, S], MDT)
        vhT = big.tile([64, S], f32)   # f32: PE transpose needs 4-byte PSUM
        project(qT, wq_sb, 128,
                [(0, 64, qh0, bq_sb[0:64, :]), (64, 128, qh1, bq_sb[64:128, :])],
                nc.sync)
        project(kT, wk_sb, 64, [(0, 64, khT, bk_sb)], nc.sync)
        project(vT, wv_sb, 64, [(0, 64, vhT, bv_sb)], nc.gpsimd)

        # ---- RoPE (in-place; pairs are partition blocks [0:32)/[32:64)) ----
        def rope64(x, nm):
            swap = stream.tile([64, S], MDT, tag="swap", name=f"swap_{nm}", bufs=2)
            for (srcp, dstp) in ((32, 0), (0, 32)):
                nc.vector.tensor_copy(swap[dstp:dstp + 32, :], x[srcp:srcp + 32, :])
            nc.vector.tensor_mul(x, x, cosk_sb)
            nc.vector.tensor_mul(swap, swap, sink_sb)
            nc.vector.tensor_add(x, x, swap)

        rope64(khT, "k")
        rope64(qh0, "q0")
        rope64(qh1, "q1")

        # ---- transpose V to [seq, dim] (+ ones column for softmax denom) ----
        vh_aug = big.tile([128, NJ, 65], MDT)
        nc.scalar.copy(vh_aug[:, :, 64], ones4q[0:128, 0:NJ])
        with nc.allow_low_precision(reason="transpose psum, same width as f32"):
            for jt in range(NJ):
                tp = psum.tile([128, 64], f32, tag="mm", bufs=4, name="tp")
                nc.tensor.transpose(tp, vhT[:, jt * JTS:(jt + 1) * JTS], ident)
                nc.scalar.copy(vh_aug[:, jt, 0:64], tp)

        # ---- attention (2 heads share khT / vh_aug) ----
        attn = big.tile([128, S], MDT)

        def out_proj(it):
            # partial over this core's 128 dims, one i-tile
            for e in range(NE):
                pw = psum.tile([128, ST], f32, tag="mm", bufs=4, name="pw")
                mm(pw, wo_sb[:, e * 128:(e + 1) * 128],
                   attn[:, it * ST:(it + 1) * ST], start=True, stop=True)
                ob = outb.tile([128, ST], MDT, tag="ob", bufs=4)
                # split psum->sbuf casts between ACT and DVE queues
                if e % 2 == 0:
                    nc.vector.tensor_copy(ob, pw)
                else:
                    nc.scalar.copy(ob, pw)
                nc.gpsimd.dma_start(
                    out=outT[e * 128:(e + 1) * 128, it * ST:(it + 1) * ST], in_=ob
                )

        for h in range(2):
            q_sl = (qh0, qh1)[h]
            po = [psum.tile([65, ST], f32, tag="acc", bufs=4, name=f"po{s}")
                  for s in range(NS)]

            def finish_pair(p):
                # its (2p, 2p+1) accumulators are complete: copy out of PSUM
                # first (frees banks + keeps PE queue unblocked), then recip
                # via DVE and broadcast via a DRAM bounce (off the PE queue).
                its = (2 * p, 2 * p + 1)
                hp = 2 * h + p
                sums = small.tile([1, 2 * ST], f32, tag="sums", name=f"sums{h}{p}")
                rc = small.tile([1, 2 * ST], f32, tag="rc", name=f"rc{h}{p}")
                for it in its:
                    c0 = ST * (it % 2)
                    nc.scalar.copy(sums[0:1, c0:c0 + ST], po[it][64:65, :])
                    a_sl = attn[h * 64:(h + 1) * 64, it * ST:(it + 1) * ST]
                    nc.scalar.copy(a_sl, po[it][0:64, :])
                nc.vector.reciprocal_approx_fast(out=rc, in_=sums)
                nc.sync.dma_start(out=rcb_d[hp:hp + 1, :], in_=rc)
                for it in its:
                    c0 = ST * (it % 2)
                    a_sl = attn[h * 64:(h + 1) * 64, it * ST:(it + 1) * ST]
                    bct = ptile.tile([128, ST], f32, tag="bct", bufs=2, name="bct")
                    bslc = bct[h * 64:(h + 1) * 64, :]
                    rsrc = rcb_d[hp:hp + 1, c0:c0 + ST]
                    rsrc = bass.AP(tensor=rsrc.tensor, offset=rsrc.offset,
                                   ap=[[0, 64]] + list(rsrc.ap)[1:])
                    nc.sync.dma_start(out=bslc, in_=rsrc)
                    nc.vector.tensor_mul(a_sl, a_sl, bslc)

            for jt in range(NJ):
                it0 = jt // 4
                for it in range(it0, NS):
                    # on the diagonal i-tile, columns below the diagonal are
                    # fully masked: skip them in scores/exp/PV entirely
                    lo = (jt - 4 * it) * JTS if it == it0 else 0
                    ps = psum.tile([128, ST], f32, tag="mm", bufs=4)
                    mm(ps[:, lo:], khT[:, jt * JTS:(jt + 1) * JTS],
                       q_sl[:, it * ST + lo:(it + 1) * ST], start=True, stop=True)
                    pt = ptile.tile([128, ST], MDT, tag="pt", bufs=6)
                    nc.scalar.activation(out=pt[:, lo:], in_=ps[:, lo:],
                                         func=Exp, scale=0.125)
                    if it == it0:
                        # partial band: keep iff jp <= c within the tile
                        nc.vector.tensor_mul(pt[:, lo:lo + JTS],
                                             pt[:, lo:lo + JTS],
                                             cm_sb)
                    mm(po[it][:, lo:], vh_aug[:, jt, :], pt[:, lo:],
                       start=(jt == 0), stop=(jt == 4 * it + 3))
                if jt == 7:
                    finish_pair(0)
                # emit early out-projections a few tiles after the normalize
                # chain kicked off, so the in-order PE queue never waits on it
                if h == 1 and jt == 10:
                    out_proj(0)
                if h == 1 and jt == 12:
                    out_proj(1)
            finish_pair(1)
        out_proj(2)
        out_proj(3)

    nc.compile()
    _CACHE["nc"] = nc
    return nc


def _host_tables():
    if "tables" in _CACHE:
        return _CACHE["tables"]
    # faithful to reference: exp = -2*arange(0,64,2)/64
    expv = -2.0 * np.arange(0, HD, 2, dtype=np.float32) / HD
    thetas = np.power(np.float32(10000.0), expv).astype(np.float32)    # [32]
    m = np.arange(S, dtype=np.float32)
    freq = np.outer(m, thetas).astype(np.float32)                      # [S, 32]
    cos = np.cos(freq).astype(np.float32).T                            # [32, S]
    sin = np.sin(freq).astype(np.float32).T
    cos64 = np.concatenate([cos, cos], 0)                              # [64, S]
    sin64 = np.concatenate([-sin, sin], 0)                             # [64, S]
    cos64 = np.ascontiguousarray(cos64).astype(NPDT)
    sin64 = np.ascontiguousarray(sin64).astype(NPDT)
    perm = np.concatenate([np.arange(0, HD, 2), np.arange(1, HD, 2)])  # deinterleave
    tril = (np.arange(128)[:, None] <= np.arange(128)[None, :])
    tril = np.ascontiguousarray(tril.astype(NPDT))
    _CACHE["tables"] = (cos64, sin64, perm, tril)
    return _CACHE["tables"]


def kernel(**inputs):
    q = np.asarray(inputs["q"], np.float32)[0]       # [S, D]
    k = np.asarray(inputs["k"], np.float32)[0]
    v = np.asarray(inputs["v"], np.float32)[0]
    Wq = np.asarray(inputs["Wq"], np.float32)
    Wk = np.asarray(inputs["Wk"], np.float32)
    Wv = np.asarray(inputs["Wv"], np.float32)
    Wo = np.asarray(inputs["Wo"], np.float32)
    bq = np.asarray(inputs["bq"], np.float32)
    bk = np.asarray(inputs["bk"], np.float32)
    bv = np.asarray(inputs["bv"], np.float32)
    bo = np.asarray(inputs["bo"], np.float32)

    cos64, sin64, perm, tril = _host_tables()

    # head_dim deinterleave permutation applied to q/k projection columns
    permQ = np.concatenate([h * HD + perm for h in range(HEADS)])
    permK = np.concatenate([g * HD + perm for g in range(KVH)])
    Wqp = Wq[:, permQ]
    bqp = bq[permQ]
    Wkp = Wk[:, permK]
    bkp = bk[permK]

    qT = np.ascontiguousarray(q.T).astype(NPDT)
    kT = np.ascontiguousarray(k.T).astype(NPDT)
    vT = np.ascontiguousarray(v.T).astype(NPDT)

    in_maps = []
    for c in range(N_CORES):
        g = c // 2
        if c % 2 == 0:
            h0, h1 = g, g + 4
        else:
            h0, h1 = g + 8, g + 12
        wq_c = np.ascontiguousarray(
            np.concatenate([Wqp[:, h0 * HD:(h0 + 1) * HD],
                            Wqp[:, h1 * HD:(h1 + 1) * HD]], axis=1))
        bq_c = np.ascontiguousarray(
            np.concatenate([bqp[h0 * HD:(h0 + 1) * HD],
                            bqp[h1 * HD:(h1 + 1) * HD]]).reshape(128, 1))
        wo_c = np.ascontiguousarray(
            np.concatenate([Wo[h0 * HD:(h0 + 1) * HD, :],
                            Wo[h1 * HD:(h1 + 1) * HD, :]], axis=0))

        def warr(w):
            # [1024, nd] -> [128, NF*nd] with chunk-of-128-rows as middle dim
            nd = w.shape[1]
            return np.ascontiguousarray(
                w.reshape(NF, FP, nd).transpose(1, 0, 2).reshape(FP, NF * nd)
            ).astype(NPDT)

        in_maps.append({
            "qT": qT, "kT": kT, "vT": vT,
            "wq": warr(wq_c),
            "wk": warr(Wkp[:, g * HD:(g + 1) * HD]),
            "wv": warr(Wv[:, g * HD:(g + 1) * HD]),
            "wo": wo_c.astype(NPDT),
            "bq": bq_c,
            "bk": np.ascontiguousarray(bkp[g * HD:(g + 1) * HD].reshape(64, 1)),
            "bv": np.ascontiguousarray(bv[g * HD:(g + 1) * HD].reshape(64, 1)),
            "cosk": cos64, "sink": sin64, "cmask": tril,
            "ident": np.eye(64, dtype=np.float32),
            "ones": np.ones((128, 128), NPDT),
        })

    nc = _build_program()
    res = run_bass_kernel_spmd(nc, in_maps, list(range(N_CORES)))
    acc = np.zeros((D, S), np.float32)
    for r in res.results:
        acc += np.asarray(r["outT"], np.float32)
    out = acc.T + bo[None, :]
    return out[None].astype(np.float32)
